# revision 10
# baseline (speedup 1.0000x reference)
"""TRN2 Bass kernel for nn_Encoder_60112362275061 (GRU encoder).

B=128, T=1024, X=256, H=512 GRU; returns final hidden state h_T [B, H].
Data-parallel over 8 NeuronCores (16 batch rows per core); weights
replicated. See build_kernel() docstring for the per-core design.

Self-contained: hardcodes shapes/sharding; only imports the container
toolchain (concourse) and numpy.
"""

import sys

for _p in ("/opt/trn_rl_repo",):
    if _p not in sys.path:
        sys.path.insert(0, _p)

import numpy as np

import concourse.bass as bass
import concourse.mybir as mybir
from concourse.tile import TileContext, add_dep_helper

F32 = mybir.dt.float32
BF16 = mybir.dt.bfloat16
F16 = mybir.dt.float16

B, T_FULL, X, H = 128, 1024, 256, 512
NCORES = 8
BS = B // NCORES          # 16 batch rows per core
NG = 4                    # h chunks == psum column groups
HC = H // NG              # 128 h dims per chunk
GFD = 3 * HC              # 384 wih cols per group [r_j|z_j|n_j]
CH = 64                   # timesteps per For_i iteration
LOOKAHEAD = 3             # steps of x-side matmul prefill on the PE


def gate_perm():
    """Permutation P of the 3H gate dim: group j gets [r_j | z_j | n_j]."""
    idx = []
    for j in range(NG):
        idx.extend(range(j * HC, (j + 1) * HC))                  # r_j
        idx.extend(range(H + j * HC, H + (j + 1) * HC))          # z_j
        idx.extend(range(2 * H + j * HC, 2 * H + (j + 1) * HC))  # n_j
    return np.array(idx)


def host_prepare_weights(W_ih, W_hh, b_ih, b_hh):
    """Device weight tensors (shared by all cores).

    wpack [128, 2*3H + 384 + 128 + 128] f16:
        wih row-chunk 0 | wih row-chunk 1 | rows 0:4 hold
        biasA4 [4,384] ([comb_r|comb_z|bih_n] per group),
        biasB4 [4,128] (bhh_n per group), ind4 [4,128].
    whhb [128, 4*1024 + 4*512 + 128] bf16: the 4 contraction chunks of
        permuted W_hh^T split into rz (256 cols/group) and n (128
        cols/group) column sets, then a 128x128 identity.
    """
    P = gate_perm()
    import ml_dtypes
    wih = np.ascontiguousarray(W_ih.T[:, P]).astype(np.float32)  # [256, 1536]
    whh = np.ascontiguousarray(W_hh.T[:, P]).astype(np.float32)  # [512, 1536]
    bih_p = b_ih[P].astype(np.float32)
    bhh_p = b_hh[P].astype(np.float32)
    comb = bih_p + bhh_p
    # psum bank A cols per step: [r | z | xn]; bank B cols: [hn]
    biasA = np.zeros((4, GFD), np.float32)
    biasB = np.zeros((4, HC), np.float32)
    for j in range(NG):
        g = j * GFD
        biasA[j, 0:2 * HC] = comb[g:g + 2 * HC]                  # r|z combined
        biasA[j, 2 * HC:3 * HC] = bih_p[g + 2 * HC:g + 3 * HC]   # xn bias
        biasB[j, 0:HC] = bhh_p[g + 2 * HC:g + 3 * HC]            # hn bias
    ind4 = np.zeros((4, 128), np.float32)
    for j in range(NG):
        ind4[j, 32 * j:32 * (j + 1)] = 1.0
    bp = np.zeros((128, GFD + HC + 128), np.float32)
    bp[0:4, 0:GFD] = biasA
    bp[0:4, GFD:GFD + HC] = biasB
    bp[0:4, GFD + HC:GFD + HC + 128] = ind4
    wpack = np.concatenate([wih[0:128], wih[128:256], bp], axis=1)
    # whh column split: per k-chunk c, rz cols [r_j|z_j]*4 then n cols [n_j]*4
    rz_sel = np.concatenate([np.arange(j * GFD, j * GFD + 2 * HC)
                             for j in range(NG)])
    n_sel = np.concatenate([np.arange(j * GFD + 2 * HC, (j + 1) * GFD)
                            for j in range(NG)])
    ident = np.eye(128, dtype=np.float32)
    parts = [whh[128 * c:128 * (c + 1)][:, rz_sel] for c in range(4)]
    parts += [whh[128 * c:128 * (c + 1)][:, n_sel] for c in range(4)]
    parts += [ident]
    whhb = np.concatenate(parts, axis=1).astype(ml_dtypes.bfloat16)
    return {"wpack": np.ascontiguousarray(wpack.astype(np.float16)),
            "whhb": np.ascontiguousarray(whhb)}


def host_prepare_x(x, core):
    """Per-core transposed x: [256, T*BS], col = t*BS + b."""
    xs = x[core * BS:(core + 1) * BS]                # [BS, T, X]
    t = xs.shape[1]
    return np.ascontiguousarray(
        xs.transpose(2, 1, 0).reshape(X, t * BS)).astype(np.float32)


def host_blob(x, wpack, core):
    """Per-core fp16 input blob: x halves then wpack (wih + biases)."""
    xt = host_prepare_x(x, core).astype(np.float16)   # [256, T*BS]
    return np.ascontiguousarray(
        np.concatenate([xt[0:128], xt[128:256], wpack], axis=1))


def host_post(out_core):
    """[112, 128] packed h' -> [BS, H]."""
    out_core = np.asarray(out_core, dtype=np.float32)
    h = np.zeros((BS, H), np.float32)
    for j in range(NG):
        h[:, j * HC:(j + 1) * HC] = out_core[32 * j:32 * j + BS, :]
    return h


def build_kernel(T=T_FULL, CH=CH):
    """Per-core GRU program.

    Packed layout: batch rows at partitions 32j+b (h-chunk j, b<16);
    rows 32j+16..32j+32 are computed junk. Two PSUM banks per step:
    bank A holds [r|z|xn] (384 f32), bank B holds [hn] (128 f32); each
    is seeded by an indicator-matrix bias matmul (start=True), bank A
    additionally accumulates the x-side matmuls. The recurrent matmuls
    run as 4 tile_position column strips x 4 K-waves, the r|z waves
    first so the sigmoid (ScalarE, psum->sbuf bf16) overlaps the hn
    waves. The tail runs in bf16 where SBUF-only (DVE 2x mode):
    m=sig(r)*hn, a=m+xn, n=tanh(a) [ACT], nwn=(z-1)*n [fused STT],
    h'=q-nwn with q=z*h computed off-chain; h' is written once as
    bf16, PE-transposed, and copied back to SBUF on ScalarE to become
    the next step's stationary lhsT. The x-side matmuls of step s+2
    are emitted right after step s's recurrent waves so the in-order
    PE stream stays busy during the elementwise tail (keeps the HAM
    clock gate at 8/8).
    """
    assert T % CH == 0 and CH % 2 == 0
    nc = bass.Bass("TRN2")

    WCOLS = 2 * 3 * H + GFD + HC + 128
    WHH_RZ = 4 * (2 * HC) * NG        # 4096
    WHH_N = 4 * HC * NG               # 2048
    xpack = nc.dram_tensor("xpack", [128, 2 * T * BS + WCOLS], F16,
                           kind="ExternalInput")
    whhb = nc.dram_tensor("whhb", [128, WHH_RZ + WHH_N + 128], BF16,
                          kind="ExternalInput")
    hout = nc.dram_tensor("hout", [112, HC], F32, kind="ExternalOutput")

    sig = mybir.ActivationFunctionType.Sigmoid
    tanh = mybir.ActivationFunctionType.Tanh

    with TileContext(nc) as tc:
        with (
            tc.tile_pool(name="consts", bufs=1) as cpool,
            tc.tile_pool(name="state", bufs=1) as spool,
            tc.tile_pool(name="xc", bufs=2) as xpool,
            tc.tile_pool(name="work", bufs=2) as wpool,
            tc.tile_pool(name="psumA", bufs=3, space="PSUM") as papool,
            tc.tile_pool(name="psumB", bufs=3, space="PSUM") as pbpool,
            tc.tile_pool(name="psumT", bufs=2, space="PSUM") as ptpool,
        ):
            # ---- resident constants + full x preload ----
            bl_sb = cpool.tile([128, 2 * T * BS + WCOLS], F16, tag="xpack")
            wh_sb = cpool.tile([128, WHH_RZ + WHH_N + 128], BF16, tag="whhb")
            nc.sync.dma_start(out=bl_sb[:], in_=xpack[:, :])
            nc.sync.dma_start(out=wh_sb[:], in_=whhb[:, :])
            xbig = bl_sb[:, 0:2 * T * BS].rearrange("p (a w) -> p a w", a=2)
            wp_sb = bl_sb[:, 2 * T * BS:]
            wih0 = wp_sb[:, 0:3 * H]
            wih1 = wp_sb[:, 3 * H:6 * H]
            bA_sb = wp_sb[0:4, 6 * H:6 * H + GFD]
            bB_sb = wp_sb[0:4, 6 * H + GFD:6 * H + GFD + HC]
            i4_sb = wp_sb[0:4, 6 * H + GFD + HC:6 * H + GFD + HC + 128]
            whh_rz = [wh_sb[:, (2 * HC) * NG * c:(2 * HC) * NG * (c + 1)]
                      for c in range(4)]
            whh_n = [wh_sb[:, WHH_RZ + HC * NG * c:WHH_RZ + HC * NG * (c + 1)]
                     for c in range(4)]
            id_bf = wh_sb[:, WHH_RZ + WHH_N:WHH_RZ + WHH_N + 128]

            # ---- persistent state (parity-indexed) ----
            hbf = [spool.tile([128, HC], BF16, tag=f"hbf{p}", name=f"hbf{p}")
                   for p in range(2)]
            hT_sb = [spool.tile([128, 128], BF16, tag=f"hT{p}", name=f"hT{p}")
                     for p in range(2)]
            # t=0 reads parity 1 (h(-1) == 0)
            nc.vector.memset(hbf[1][:], 0.0)
            nc.vector.memset(hT_sb[1][:], 0.0)

            # prime the sigmoid/tanh table set before the loop
            prime = wpool.tile([128, 2], F32, tag="prime")
            nc.scalar.activation(prime[:, 0:1], wp_sb[:, 0:1], sig)
            nc.scalar.activation(prime[:, 1:2], prime[:, 0:1], tanh)

            psum_tiles = {}  # step -> (pA, pB), filled LOOKAHEAD early

            def emit_xside(s, xc0, xc1):
                """Bias + x-side matmuls filling psum banks of step s."""
                pA = papool.tile([128, GFD], F32, tag="pA")
                pB = pbpool.tile([128, HC], F32, tag="pB")
                psum_tiles[s] = (pA, pB)
                sl = bass.ts(s, BS)
                nc.tensor.matmul(pA[:, :], i4_sb, bA_sb,
                                 start=True, stop=False, tile_position=(0, 0),
                                 skip_group_check=True)
                nc.tensor.matmul(pB[:, :], i4_sb, bB_sb,
                                 start=True, stop=False, tile_position=(0, 0),
                                 skip_group_check=True)
                for j in range(NG):
                    o = slice(32 * j, 32 * j + BS)
                    g0 = j * GFD
                    nc.tensor.matmul(pA[o, :], xc0[:, sl],
                                     wih0[:, g0:g0 + GFD],
                                     start=False, stop=False,
                                     tile_position=(0, 32 * j),
                                     skip_group_check=True)
                    nc.tensor.matmul(pA[o, :], xc1[:, sl],
                                     wih1[:, g0:g0 + GFD],
                                     start=False, stop=False,
                                     tile_position=(0, 32 * j),
                                     skip_group_check=True)

            def step(s, prefill):
                """Emit one timestep; prefill emits step s+LOOKAHEAD's
                x-side matmuls right after the recurrent waves."""
                p = s % 2
                pA, pB = psum_tiles.pop(s)

                # --- transpose h(s-1) -> hT (PE), copy to SBUF on ACT ---
                pT = ptpool.tile([128, 128], BF16, tag="pT")
                nc.tensor.transpose(pT[:, :], hbf[1 - p][:, :], id_bf)
                nc.vector.tensor_copy(hT_sb[1 - p][:, :], pT[:, :])

                # --- recurrent waves: r|z first (bank A), then hn (bank B) ---
                for c in range(4):
                    for j in range(NG):
                        oo = slice(32 * j, 32 * (j + 1))
                        nc.tensor.matmul(
                            pA[oo, 0:2 * HC],
                            hT_sb[1 - p][:, 32 * c:32 * (c + 1)],
                            whh_rz[c][:, j * 2 * HC:(j + 1) * 2 * HC],
                            start=False, stop=(c == 3 and j == NG - 1),
                            tile_position=(0, 32 * j),
                            skip_group_check=True)
                for c in range(4):
                    for j in range(NG):
                        oo = slice(32 * j, 32 * (j + 1))
                        nc.tensor.matmul(
                            pB[oo, :],
                            hT_sb[1 - p][:, 32 * c:32 * (c + 1)],
                            whh_n[c][:, j * HC:(j + 1) * HC],
                            start=False, stop=(c == 3 and j == NG - 1),
                            tile_position=(0, 32 * j),
                            skip_group_check=True)

                # --- PE prefill: x-side matmuls of step s+LOOKAHEAD ---
                if prefill is not None:
                    emit_xside(*prefill)

                # --- elementwise tail ---
                sr = wpool.tile([128, HC], BF16, tag="sr")
                sz = wpool.tile([128, HC], BF16, tag="sz")
                m = wpool.tile([128, HC], BF16, tag="m")
                a = wpool.tile([128, HC], BF16, tag="a")
                n_t = wpool.tile([128, HC], BF16, tag="n")
                q = wpool.tile([128, HC], BF16, tag="q")
                nwn = wpool.tile([128, HC], BF16, tag="nwn")

                nc.scalar.activation(sr[:], pA[:, 0:HC], sig)
                nc.scalar.activation(sz[:], pA[:, HC:2 * HC], sig)
                mi = nc.vector.tensor_tensor(m[:], sr[:], pB[:, 0:HC],
                                             mybir.AluOpType.mult)
                ai = nc.vector.tensor_tensor(a[:], m[:], pA[:, 2 * HC:GFD],
                                             mybir.AluOpType.add)
                nc.scalar.activation(n_t[:], a[:], tanh)
                # q = z * h(s-1) runs on DVE during the tanh; the explicit
                # dep keeps the scheduler from hoisting it ahead of m/a
                qi = nc.vector.tensor_tensor(q[:], sz[:], hbf[1 - p][:, :],
                                             mybir.AluOpType.mult)
                add_dep_helper(qi.ins, ai.ins, sync=False,
                               reason="q runs after a")
                # nwn = (z - 1) * n ; h' = q - nwn = z*h + (1-z)*n
                nc.vector.scalar_tensor_tensor(nwn[:], sz[:], 1.0, n_t[:],
                                               mybir.AluOpType.subtract,
                                               mybir.AluOpType.mult)
                nc.vector.tensor_tensor(hbf[p][:, :], q[:], nwn[:],
                                        mybir.AluOpType.subtract)

            if T == CH:
                xc01 = (xbig[:, 0, 0:CH * BS], xbig[:, 1, 0:CH * BS])
                for s in range(LOOKAHEAD):
                    emit_xside(s, *xc01)
                for s in range(CH):
                    pre = ((s + LOOKAHEAD, *xc01)
                           if s + LOOKAHEAD < CH else None)
                    step(s, pre)
            else:
                with tc.For_i(0, T * BS, CH * BS,
                              hint_engines=tuple(mybir.ALL_ENGINES)) as iv:
                    # chunk copy resolves the dynamic offset (ldweights
                    # cannot take register offsets)
                    xc = xpool.tile([128, 2, CH * BS], F16, tag="xc")
                    nc.vector.tensor_copy(
                        xc[:, :, :], xbig[:, :, bass.ds(iv, CH * BS)])
                    xc01 = (xc[:, 0, :], xc[:, 1, :])
                    # steps 0..LOOKAHEAD-1 get their x matmuls up front
                    for s in range(LOOKAHEAD):
                        emit_xside(s, *xc01)
                    for s in range(CH):
                        pre = ((s + LOOKAHEAD, *xc01)
                               if s + LOOKAHEAD < CH else None)
                        step(s, pre)

            # final h lives in hbf[(T-1) % 2]; cast to f32 and store
            hf = wpool.tile([112, HC], F32, tag="hf")
            nc.vector.tensor_copy(hf[:, :], hbf[(T - 1) % 2][0:112, :])
            nc.sync.dma_start(out=hout[:, :], in_=hf[:, :])

    _split_sync_waits(nc)
    return nc


def _split_sync_waits(nc):
    """Walrus codegen allows exactly ONE sync wait per instruction (the TPB
    events struct has a single wait slot). Tile emits multi-wait
    instructions (loop back-edge drains, barrier NoOps, cross-engine RAW
    joins); split the extras onto same-engine NoOps inserted immediately
    before -- the sequencer processes them in order, so semantics are
    identical."""
    for blk in nc.m.functions[0].blocks:
        i = 0
        while i < len(blk.instructions):
            inst = blk.instructions[i]
            si = getattr(inst, "sync_info", None)
            if si and si.on_wait and len(si.on_wait) > 1:
                waits = list(si.on_wait)
                si.on_wait = [waits[-1]]
                for w in waits[:-1]:
                    nop = mybir.InstNoOp(
                        name=nc.get_next_instruction_name(), ins=[], outs=[])
                    nop.engine = inst.engine
                    nop.sync_info = mybir.SyncInfo(on_wait=[w], on_update=[])
                    nc.register_instruction(nop)
                    blk.instructions.insert(i, nop)
                    i += 1
            i += 1


_NC_CACHE = {}


def run(x, W_ih, W_hh, b_ih, b_hh, trace=False):
    from concourse.bass_utils import run_bass_kernel_spmd

    x = np.asarray(x, dtype=np.float32)
    W_ih = np.asarray(W_ih, dtype=np.float32)
    W_hh = np.asarray(W_hh, dtype=np.float32)
    b_ih = np.asarray(b_ih, dtype=np.float32)
    b_hh = np.asarray(b_hh, dtype=np.float32)

    key = (x.shape[1],)
    if key not in _NC_CACHE:
        _NC_CACHE[key] = build_kernel(T=x.shape[1])
    nc = _NC_CACHE[key]

    wts = host_prepare_weights(W_ih, W_hh, b_ih, b_hh)
    in_maps = [{"xpack": host_blob(x, wts["wpack"], c), "whhb": wts["whhb"]}
               for c in range(NCORES)]
    res = run_bass_kernel_spmd(nc, in_maps, list(range(NCORES)), trace=trace)
    h = np.zeros((B, H), np.float32)
    for c in range(NCORES):
        h[c * BS:(c + 1) * BS] = host_post(np.asarray(res.results[c]["hout"]))
    return h, res


def kernel(x, W_ih, W_hh, b_ih, b_hh):
    h, _ = run(x, W_ih, W_hh, b_ih, b_hh)
    return h


# revision 13
# speedup vs baseline: 1.0696x; 1.0696x over previous
"""TRN2 Bass kernel for nn_Encoder_60112362275061 (GRU encoder).

B=128, T=1024, X=256, H=512 GRU; returns final hidden state h_T [B, H].
Data-parallel over 8 NeuronCores (16 batch rows per core); weights
replicated. See build_kernel() docstring for the per-core design.

Self-contained: hardcodes shapes/sharding; only imports the container
toolchain (concourse) and numpy.
"""

import sys

for _p in ("/opt/trn_rl_repo",):
    if _p not in sys.path:
        sys.path.insert(0, _p)

import numpy as np

import concourse.bass as bass
import concourse.mybir as mybir
from concourse.tile import TileContext, add_dep_helper

F32 = mybir.dt.float32
BF16 = mybir.dt.bfloat16
F16 = mybir.dt.float16

B, T_FULL, X, H = 128, 1024, 256, 512
NCORES = 8
BS = B // NCORES          # 16 batch rows per core
NG = 4                    # h chunks == psum column groups
HC = H // NG              # 128 h dims per chunk
GFD = 3 * HC              # 384 wih cols per group [r_j|z_j|n_j]
CH = 64                   # timesteps per For_i iteration
LOOKAHEAD = 2             # steps of x-side matmul prefill on the PE
NDUM = 6                  # dummy matmuls per step keeping the PE HAM-warm


def gate_perm():
    """Permutation P of the 3H gate dim: group j gets [r_j | z_j | n_j]."""
    idx = []
    for j in range(NG):
        idx.extend(range(j * HC, (j + 1) * HC))                  # r_j
        idx.extend(range(H + j * HC, H + (j + 1) * HC))          # z_j
        idx.extend(range(2 * H + j * HC, 2 * H + (j + 1) * HC))  # n_j
    return np.array(idx)


def host_prepare_weights(W_ih, W_hh, b_ih, b_hh):
    """Device weight tensors (shared by all cores).

    wpack [128, 2*3H + 384 + 128 + 128] f16:
        wih row-chunk 0 | wih row-chunk 1 | rows 0:4 hold
        biasA4 [4,384] ([comb_r|comb_z|bih_n] per group),
        biasB4 [4,128] (bhh_n per group), ind4 [4,128].
    whhb [128, 4*1024 + 4*512 + 128] bf16: the 4 contraction chunks of
        permuted W_hh^T split into rz (256 cols/group) and n (128
        cols/group) column sets, then a 128x128 identity.
    """
    P = gate_perm()
    import ml_dtypes
    wih = np.ascontiguousarray(W_ih.T[:, P]).astype(np.float32)  # [256, 1536]
    whh = np.ascontiguousarray(W_hh.T[:, P]).astype(np.float32)  # [512, 1536]
    bih_p = b_ih[P].astype(np.float32)
    bhh_p = b_hh[P].astype(np.float32)
    comb = bih_p + bhh_p
    # psum bank A cols per step: [r | z | xn]; bank B cols: [hn]
    biasA = np.zeros((4, GFD), np.float32)
    biasB = np.zeros((4, HC), np.float32)
    for j in range(NG):
        g = j * GFD
        biasA[j, 0:2 * HC] = comb[g:g + 2 * HC]                  # r|z combined
        biasA[j, 2 * HC:3 * HC] = bih_p[g + 2 * HC:g + 3 * HC]   # xn bias
        biasB[j, 0:HC] = bhh_p[g + 2 * HC:g + 3 * HC]            # hn bias
    ind4 = np.zeros((4, 128), np.float32)
    for j in range(NG):
        ind4[j, 32 * j:32 * (j + 1)] = 1.0
    bp = np.zeros((128, GFD + HC + 128), np.float32)
    bp[0:4, 0:GFD] = biasA
    bp[0:4, GFD:GFD + HC] = biasB
    bp[0:4, GFD + HC:GFD + HC + 128] = ind4
    wpack = np.concatenate([wih[0:128], wih[128:256], bp], axis=1)
    # whh column split: per k-chunk c, rz cols [r_j|z_j]*4 then n cols [n_j]*4
    rz_sel = np.concatenate([np.arange(j * GFD, j * GFD + 2 * HC)
                             for j in range(NG)])
    n_sel = np.concatenate([np.arange(j * GFD + 2 * HC, (j + 1) * GFD)
                            for j in range(NG)])
    ident = np.eye(128, dtype=np.float32)
    parts = [whh[128 * c:128 * (c + 1)][:, rz_sel] for c in range(4)]
    parts += [whh[128 * c:128 * (c + 1)][:, n_sel] for c in range(4)]
    parts += [ident]
    whhb = np.concatenate(parts, axis=1).astype(ml_dtypes.bfloat16)
    return {"wpack": np.ascontiguousarray(wpack.astype(np.float16)),
            "whhb": np.ascontiguousarray(whhb)}


def host_prepare_x(x, core):
    """Per-core transposed x: [256, T*BS], col = t*BS + b."""
    xs = x[core * BS:(core + 1) * BS]                # [BS, T, X]
    t = xs.shape[1]
    return np.ascontiguousarray(
        xs.transpose(2, 1, 0).reshape(X, t * BS)).astype(np.float32)


def host_blob(x, wpack, core):
    """Per-core fp16 input blob: x halves then wpack (wih + biases)."""
    xt = host_prepare_x(x, core).astype(np.float16)   # [256, T*BS]
    return np.ascontiguousarray(
        np.concatenate([xt[0:128], xt[128:256], wpack], axis=1))


def host_post(out_core):
    """[112, 128] packed h' -> [BS, H]."""
    out_core = np.asarray(out_core, dtype=np.float32)
    h = np.zeros((BS, H), np.float32)
    for j in range(NG):
        h[:, j * HC:(j + 1) * HC] = out_core[32 * j:32 * j + BS, :]
    return h


def build_kernel(T=T_FULL, CH=CH):
    """Per-core GRU program.

    Packed layout: batch rows at partitions 32j+b (h-chunk j, b<16);
    rows 32j+16..32j+32 are computed junk. Two PSUM banks per step:
    bank A holds [r|z|xn] (384 f32), bank B holds [hn] (128 f32); each
    is seeded by an indicator-matrix bias matmul (start=True), bank A
    additionally accumulates the x-side matmuls. The recurrent matmuls
    run as 4 tile_position column strips x 4 K-waves, the r|z waves
    first so the sigmoid (ScalarE, psum->sbuf bf16) overlaps the hn
    waves. The tail runs in bf16 where SBUF-only (DVE 2x mode):
    m=sig(r)*hn, a=m+xn, n=tanh(a) [ACT], nwn=(z-1)*n [fused STT],
    h'=q-nwn with q=z*h computed off-chain; h' is written once as
    bf16, PE-transposed, and copied back to SBUF on ScalarE to become
    the next step's stationary lhsT. The x-side matmuls of step s+2
    are emitted right after step s's recurrent waves so the in-order
    PE stream stays busy during the elementwise tail (keeps the HAM
    clock gate at 8/8).
    """
    assert T % CH == 0 and CH % 2 == 0
    nc = bass.Bass("TRN2")

    WCOLS = 2 * 3 * H + GFD + HC + 128
    WHH_RZ = 4 * (2 * HC) * NG        # 4096
    WHH_N = 4 * HC * NG               # 2048
    xpack = nc.dram_tensor("xpack", [128, 2 * T * BS + WCOLS], F16,
                           kind="ExternalInput")
    whhb = nc.dram_tensor("whhb", [128, WHH_RZ + WHH_N + 128], BF16,
                          kind="ExternalInput")
    hout = nc.dram_tensor("hout", [112, HC], F32, kind="ExternalOutput")

    sig = mybir.ActivationFunctionType.Sigmoid
    tanh = mybir.ActivationFunctionType.Tanh

    with TileContext(nc) as tc:
        with (
            tc.tile_pool(name="consts", bufs=1) as cpool,
            tc.tile_pool(name="state", bufs=1) as spool,
            tc.tile_pool(name="xc", bufs=2) as xpool,
            tc.tile_pool(name="work", bufs=2) as wpool,
            tc.tile_pool(name="psumA", bufs=3, space="PSUM") as papool,
            tc.tile_pool(name="psumB", bufs=3, space="PSUM") as pbpool,
            tc.tile_pool(name="psumT", bufs=2, space="PSUM") as ptpool,
        ):
            # ---- resident constants + full x preload ----
            bl_sb = cpool.tile([128, 2 * T * BS + WCOLS], F16, tag="xpack")
            wh_sb = cpool.tile([128, WHH_RZ + WHH_N + 128], BF16, tag="whhb")
            nc.sync.dma_start(out=bl_sb[:], in_=xpack[:, :])
            nc.sync.dma_start(out=wh_sb[:], in_=whhb[:, :])
            xbig = bl_sb[:, 0:2 * T * BS].rearrange("p (a w) -> p a w", a=2)
            wp_sb = bl_sb[:, 2 * T * BS:]
            wih0 = wp_sb[:, 0:3 * H]
            wih1 = wp_sb[:, 3 * H:6 * H]
            bA_sb = wp_sb[0:4, 6 * H:6 * H + GFD]
            bB_sb = wp_sb[0:4, 6 * H + GFD:6 * H + GFD + HC]
            i4_sb = wp_sb[0:4, 6 * H + GFD + HC:6 * H + GFD + HC + 128]
            whh_rz = [wh_sb[:, (2 * HC) * NG * c:(2 * HC) * NG * (c + 1)]
                      for c in range(4)]
            whh_n = [wh_sb[:, WHH_RZ + HC * NG * c:WHH_RZ + HC * NG * (c + 1)]
                     for c in range(4)]
            id_bf = wh_sb[:, WHH_RZ + WHH_N:WHH_RZ + WHH_N + 128]

            # ---- persistent state (parity-indexed) ----
            hbf = [spool.tile([128, HC], BF16, tag=f"hbf{p}", name=f"hbf{p}")
                   for p in range(2)]
            hT_sb = [spool.tile([128, 128], BF16, tag=f"hT{p}", name=f"hT{p}")
                     for p in range(2)]
            # t=0 reads parity 1 (h(-1) == 0)
            nc.vector.memset(hbf[1][:], 0.0)
            nc.vector.memset(hT_sb[1][:], 0.0)

            # prime the sigmoid/tanh table set before the loop
            prime = wpool.tile([128, 2], F32, tag="prime")
            nc.scalar.activation(prime[:, 0:1], wp_sb[:, 0:1], sig)
            nc.scalar.activation(prime[:, 1:2], prime[:, 0:1], tanh)

            psum_tiles = {}  # step -> (pA, pB), filled LOOKAHEAD early

            def emit_xside(s, xc0, xc1):
                """Bias + x-side matmuls filling psum banks of step s."""
                pA = papool.tile([128, GFD], F32, tag="pA")
                pB = pbpool.tile([128, HC], F32, tag="pB")
                psum_tiles[s] = (pA, pB)
                sl = bass.ts(s, BS)
                nc.tensor.matmul(pA[:, :], i4_sb, bA_sb,
                                 start=True, stop=False, tile_position=(0, 0),
                                 skip_group_check=True)
                nc.tensor.matmul(pB[:, :], i4_sb, bB_sb,
                                 start=True, stop=False, tile_position=(0, 0),
                                 skip_group_check=True)
                for j in range(NG):
                    o = slice(32 * j, 32 * j + BS)
                    g0 = j * GFD
                    nc.tensor.matmul(pA[o, :], xc0[:, sl],
                                     wih0[:, g0:g0 + GFD],
                                     start=False, stop=False,
                                     tile_position=(0, 32 * j),
                                     skip_group_check=True)
                    nc.tensor.matmul(pA[o, :], xc1[:, sl],
                                     wih1[:, g0:g0 + GFD],
                                     start=False, stop=False,
                                     tile_position=(0, 32 * j),
                                     skip_group_check=True)

            def step(s, prefill):
                """Emit one timestep; prefill emits step s+LOOKAHEAD's
                x-side matmuls right after the recurrent waves."""
                p = s % 2
                pA, pB = psum_tiles.pop(s)

                # --- transpose h(s-1) -> hT (PE), copy to SBUF on ACT ---
                pT = ptpool.tile([128, 128], BF16, tag="pT")
                nc.tensor.transpose(pT[:, :], hbf[1 - p][:, :], id_bf)
                nc.vector.tensor_copy(hT_sb[1 - p][:, :], pT[:, :])

                # --- recurrent waves: r|z first (bank A), then hn (bank B) ---
                for c in range(4):
                    for j in range(NG):
                        oo = slice(32 * j, 32 * (j + 1))
                        nc.tensor.matmul(
                            pA[oo, 0:2 * HC],
                            hT_sb[1 - p][:, 32 * c:32 * (c + 1)],
                            whh_rz[c][:, j * 2 * HC:(j + 1) * 2 * HC],
                            start=False, stop=(c == 3 and j == NG - 1),
                            tile_position=(0, 32 * j),
                            skip_group_check=True)
                for c in range(4):
                    for j in range(NG):
                        oo = slice(32 * j, 32 * (j + 1))
                        nc.tensor.matmul(
                            pB[oo, :],
                            hT_sb[1 - p][:, 32 * c:32 * (c + 1)],
                            whh_n[c][:, j * HC:(j + 1) * HC],
                            start=False, stop=(c == 3 and j == NG - 1),
                            tile_position=(0, 32 * j),
                            skip_group_check=True)

                # --- PE prefill: x-side matmuls of step s+LOOKAHEAD ---
                if prefill is not None:
                    emit_xside(*prefill)

                # --- dummy matmuls: keep the PE busy through the tail so
                # the HAM clock gate stays at 8/8 (transposes don't count
                # as PE-busy; only real matmuls do). Results are junk and
                # never read; they cycle the pT psum slots.
                for k in range(NDUM):
                    dmy = ptpool.tile([32, 512], F32, tag="pT")
                    nc.tensor.matmul(dmy[:, :], wh_sb[:, 0:32],
                                     wh_sb[:, 0:512],
                                     start=True, stop=True,
                                     tile_position=(0, 0),
                                     skip_group_check=True)

                # --- elementwise tail ---
                sr = wpool.tile([128, HC], BF16, tag="sr")
                sz = wpool.tile([128, HC], BF16, tag="sz")
                m = wpool.tile([128, HC], BF16, tag="m")
                a = wpool.tile([128, HC], BF16, tag="a")
                n_t = wpool.tile([128, HC], BF16, tag="n")
                q = wpool.tile([128, HC], BF16, tag="q")
                nwn = wpool.tile([128, HC], BF16, tag="nwn")

                nc.scalar.activation(sr[:], pA[:, 0:HC], sig)
                nc.scalar.activation(sz[:], pA[:, HC:2 * HC], sig)
                mi = nc.vector.tensor_tensor(m[:], sr[:], pB[:, 0:HC],
                                             mybir.AluOpType.mult)
                ai = nc.vector.tensor_tensor(a[:], m[:], pA[:, 2 * HC:GFD],
                                             mybir.AluOpType.add)
                nc.scalar.activation(n_t[:], a[:], tanh)
                # q = z * h(s-1) runs on DVE during the tanh; the explicit
                # dep keeps the scheduler from hoisting it ahead of m/a
                qi = nc.vector.tensor_tensor(q[:], sz[:], hbf[1 - p][:, :],
                                             mybir.AluOpType.mult)
                add_dep_helper(qi.ins, ai.ins, sync=False,
                               reason="q runs after a")
                # nwn = (z - 1) * n ; h' = q - nwn = z*h + (1-z)*n
                nc.vector.scalar_tensor_tensor(nwn[:], sz[:], 1.0, n_t[:],
                                               mybir.AluOpType.subtract,
                                               mybir.AluOpType.mult)
                nc.vector.tensor_tensor(hbf[p][:, :], q[:], nwn[:],
                                        mybir.AluOpType.subtract)

            if T == CH:
                xc01 = (xbig[:, 0, 0:CH * BS], xbig[:, 1, 0:CH * BS])
                for s in range(LOOKAHEAD):
                    emit_xside(s, *xc01)
                for s in range(CH):
                    pre = ((s + LOOKAHEAD, *xc01)
                           if s + LOOKAHEAD < CH else None)
                    step(s, pre)
            else:
                with tc.For_i(0, T * BS, CH * BS,
                              hint_engines=tuple(mybir.ALL_ENGINES)) as iv:
                    # chunk copy resolves the dynamic offset (ldweights
                    # cannot take register offsets)
                    xc = xpool.tile([128, 2, CH * BS], F16, tag="xc")
                    nc.vector.tensor_copy(
                        xc[:, :, :], xbig[:, :, bass.ds(iv, CH * BS)])
                    xc01 = (xc[:, 0, :], xc[:, 1, :])
                    # steps 0..LOOKAHEAD-1 get their x matmuls up front
                    for s in range(LOOKAHEAD):
                        emit_xside(s, *xc01)
                    for s in range(CH):
                        pre = ((s + LOOKAHEAD, *xc01)
                               if s + LOOKAHEAD < CH else None)
                        step(s, pre)

            # final h lives in hbf[(T-1) % 2]; cast to f32 and store
            hf = wpool.tile([112, HC], F32, tag="hf")
            nc.vector.tensor_copy(hf[:, :], hbf[(T - 1) % 2][0:112, :])
            nc.sync.dma_start(out=hout[:, :], in_=hf[:, :])

    _split_sync_waits(nc)
    return nc


def _split_sync_waits(nc):
    """Walrus codegen allows exactly ONE sync wait per instruction (the TPB
    events struct has a single wait slot). Tile emits multi-wait
    instructions (loop back-edge drains, barrier NoOps, cross-engine RAW
    joins); split the extras onto same-engine NoOps inserted immediately
    before -- the sequencer processes them in order, so semantics are
    identical."""
    for blk in nc.m.functions[0].blocks:
        i = 0
        while i < len(blk.instructions):
            inst = blk.instructions[i]
            si = getattr(inst, "sync_info", None)
            if si and si.on_wait and len(si.on_wait) > 1:
                waits = list(si.on_wait)
                si.on_wait = [waits[-1]]
                for w in waits[:-1]:
                    nop = mybir.InstNoOp(
                        name=nc.get_next_instruction_name(), ins=[], outs=[])
                    nop.engine = inst.engine
                    nop.sync_info = mybir.SyncInfo(on_wait=[w], on_update=[])
                    nc.register_instruction(nop)
                    blk.instructions.insert(i, nop)
                    i += 1
            i += 1


_NC_CACHE = {}


def run(x, W_ih, W_hh, b_ih, b_hh, trace=False):
    from concourse.bass_utils import run_bass_kernel_spmd

    x = np.asarray(x, dtype=np.float32)
    W_ih = np.asarray(W_ih, dtype=np.float32)
    W_hh = np.asarray(W_hh, dtype=np.float32)
    b_ih = np.asarray(b_ih, dtype=np.float32)
    b_hh = np.asarray(b_hh, dtype=np.float32)

    key = (x.shape[1],)
    if key not in _NC_CACHE:
        _NC_CACHE[key] = build_kernel(T=x.shape[1])
    nc = _NC_CACHE[key]

    wts = host_prepare_weights(W_ih, W_hh, b_ih, b_hh)
    in_maps = [{"xpack": host_blob(x, wts["wpack"], c), "whhb": wts["whhb"]}
               for c in range(NCORES)]
    res = run_bass_kernel_spmd(nc, in_maps, list(range(NCORES)), trace=trace)
    h = np.zeros((B, H), np.float32)
    for c in range(NCORES):
        h[c * BS:(c + 1) * BS] = host_post(np.asarray(res.results[c]["hout"]))
    return h, res


def kernel(x, W_ih, W_hh, b_ih, b_hh):
    h, _ = run(x, W_ih, W_hh, b_ih, b_hh)
    return h


# revision 18
# speedup vs baseline: 1.0917x; 1.0206x over previous
"""TRN2 Bass kernel for nn_Encoder_60112362275061 (GRU encoder).

B=128, T=1024, X=256, H=512 GRU; returns final hidden state h_T [B, H].
Data-parallel over 8 NeuronCores (16 batch rows per core); weights
replicated. See build_kernel() docstring for the per-core design.

Self-contained: hardcodes shapes/sharding; only imports the container
toolchain (concourse) and numpy.
"""

import sys

for _p in ("/opt/trn_rl_repo",):
    if _p not in sys.path:
        sys.path.insert(0, _p)

import numpy as np

import concourse.bass as bass
import concourse.mybir as mybir
from concourse.tile import TileContext, add_dep_helper

F32 = mybir.dt.float32
BF16 = mybir.dt.bfloat16
F16 = mybir.dt.float16

B, T_FULL, X, H = 128, 1024, 256, 512
NCORES = 8
BS = B // NCORES          # 16 batch rows per core
NG = 4                    # h chunks == psum column groups
HC = H // NG              # 128 h dims per chunk
GFD = 3 * HC              # 384 wih cols per group [r_j|z_j|n_j]
CH = 64                   # timesteps per For_i iteration
LOOKAHEAD = 2             # steps of x-side matmul prefill on the PE
NDUM = 5                  # dummy matmuls per step keeping the PE HAM-warm


def gate_perm():
    """Permutation P of the 3H gate dim: group j gets [r_j | z_j | n_j]."""
    idx = []
    for j in range(NG):
        idx.extend(range(j * HC, (j + 1) * HC))                  # r_j
        idx.extend(range(H + j * HC, H + (j + 1) * HC))          # z_j
        idx.extend(range(2 * H + j * HC, 2 * H + (j + 1) * HC))  # n_j
    return np.array(idx)


def host_prepare_weights(W_ih, W_hh, b_ih, b_hh):
    """Device weight tensors (shared by all cores).

    wpack [128, 2*3H + 384 + 128 + 128] f16:
        wih row-chunk 0 | wih row-chunk 1 | rows 0:4 hold
        biasA4 [4,384] ([comb_r|comb_z|bih_n] per group),
        biasB4 [4,128] (bhh_n per group), ind4 [4,128].
    whhb [128, 4*1024 + 4*512 + 128] bf16: the 4 contraction chunks of
        permuted W_hh^T split into rz (256 cols/group) and n (128
        cols/group) column sets, then a 128x128 identity.
    """
    P = gate_perm()
    import ml_dtypes
    wih = np.ascontiguousarray(W_ih.T[:, P]).astype(np.float32)  # [256, 1536]
    whh = np.ascontiguousarray(W_hh.T[:, P]).astype(np.float32)  # [512, 1536]
    bih_p = b_ih[P].astype(np.float32)
    bhh_p = b_hh[P].astype(np.float32)
    comb = bih_p + bhh_p
    # psum bank A cols per step: [r | z | xn]; bank B cols: [hn]
    biasA = np.zeros((4, GFD), np.float32)
    biasB = np.zeros((4, HC), np.float32)
    for j in range(NG):
        g = j * GFD
        biasA[j, 0:2 * HC] = comb[g:g + 2 * HC]                  # r|z combined
        biasA[j, 2 * HC:3 * HC] = bih_p[g + 2 * HC:g + 3 * HC]   # xn bias
        biasB[j, 0:HC] = bhh_p[g + 2 * HC:g + 3 * HC]            # hn bias
    ind4 = np.zeros((4, 128), np.float32)
    for j in range(NG):
        ind4[j, 32 * j:32 * (j + 1)] = 1.0
    bp = np.zeros((128, GFD + HC + 128), np.float32)
    bp[0:4, 0:GFD] = biasA
    bp[0:4, GFD:GFD + HC] = biasB
    bp[0:4, GFD + HC:GFD + HC + 128] = ind4
    wpack = np.concatenate([wih[0:128], wih[128:256], bp], axis=1)
    # whh column split: per k-chunk c, rz cols [r_j|z_j]*4 then n cols [n_j]*4
    rz_sel = np.concatenate([np.arange(j * GFD, j * GFD + 2 * HC)
                             for j in range(NG)])
    n_sel = np.concatenate([np.arange(j * GFD + 2 * HC, (j + 1) * GFD)
                            for j in range(NG)])
    ident = np.eye(128, dtype=np.float32)
    parts = [whh[128 * c:128 * (c + 1)][:, rz_sel] for c in range(4)]
    parts += [whh[128 * c:128 * (c + 1)][:, n_sel] for c in range(4)]
    parts += [ident]
    whhb = np.concatenate(parts, axis=1).astype(ml_dtypes.bfloat16)
    return {"wpack": np.ascontiguousarray(wpack.astype(np.float16)),
            "whhb": np.ascontiguousarray(whhb)}


def host_prepare_x(x, core):
    """Per-core transposed x: [256, T*BS], col = t*BS + b."""
    xs = x[core * BS:(core + 1) * BS]                # [BS, T, X]
    t = xs.shape[1]
    return np.ascontiguousarray(
        xs.transpose(2, 1, 0).reshape(X, t * BS)).astype(np.float32)


def host_blob(x, wpack, core):
    """Per-core fp16 input blob: x halves then wpack (wih + biases)."""
    xt = host_prepare_x(x, core).astype(np.float16)   # [256, T*BS]
    return np.ascontiguousarray(
        np.concatenate([xt[0:128], xt[128:256], wpack], axis=1))


def host_post(out_core):
    """[112, 128] packed h' -> [BS, H]."""
    out_core = np.asarray(out_core, dtype=np.float32)
    h = np.zeros((BS, H), np.float32)
    for j in range(NG):
        h[:, j * HC:(j + 1) * HC] = out_core[32 * j:32 * j + BS, :]
    return h


def build_kernel(T=T_FULL, CH=CH):
    """Per-core GRU program.

    Packed layout: batch rows at partitions 32j+b (h-chunk j, b<16);
    rows 32j+16..32j+32 are computed junk. Two PSUM banks per step:
    bank A holds [r|z|xn] (384 f32), bank B holds [hn] (128 f32); each
    is seeded by an indicator-matrix bias matmul (start=True), bank A
    additionally accumulates the x-side matmuls. The recurrent matmuls
    run as 4 tile_position column strips x 4 K-waves, the r|z waves
    first so the sigmoid (ScalarE, psum->sbuf bf16) overlaps the hn
    waves. The tail runs in bf16 where SBUF-only (DVE 2x mode):
    m=sig(r)*hn, a=m+xn, n=tanh(a) [ACT], nwn=(z-1)*n [fused STT],
    h'=q-nwn with q=z*h computed off-chain; h' is written once as
    bf16, PE-transposed, and copied back to SBUF on ScalarE to become
    the next step's stationary lhsT. The x-side matmuls of step s+2
    are emitted right after step s's recurrent waves so the in-order
    PE stream stays busy during the elementwise tail (keeps the HAM
    clock gate at 8/8).
    """
    assert T % CH == 0 and CH % 2 == 0
    nc = bass.Bass("TRN2")

    WCOLS = 2 * 3 * H + GFD + HC + 128
    WHH_RZ = 4 * (2 * HC) * NG        # 4096
    WHH_N = 4 * HC * NG               # 2048
    xpack = nc.dram_tensor("xpack", [128, 2 * T * BS + WCOLS], F16,
                           kind="ExternalInput")
    whhb = nc.dram_tensor("whhb", [128, WHH_RZ + WHH_N + 128], BF16,
                          kind="ExternalInput")
    hout = nc.dram_tensor("hout", [112, HC], F32, kind="ExternalOutput")

    sig = mybir.ActivationFunctionType.Sigmoid
    tanh = mybir.ActivationFunctionType.Tanh

    with TileContext(nc) as tc:
        with (
            tc.tile_pool(name="consts", bufs=1) as cpool,
            tc.tile_pool(name="state", bufs=1) as spool,
            tc.tile_pool(name="xc", bufs=2) as xpool,
            tc.tile_pool(name="work", bufs=2) as wpool,
            tc.tile_pool(name="psumA", bufs=3, space="PSUM") as papool,
            tc.tile_pool(name="psumB", bufs=3, space="PSUM") as pbpool,
            tc.tile_pool(name="psumT", bufs=2, space="PSUM") as ptpool,
        ):
            # ---- resident constants + full x preload ----
            bl_sb = cpool.tile([128, 2 * T * BS + WCOLS], F16, tag="xpack")
            wh_sb = cpool.tile([128, WHH_RZ + WHH_N + 128], BF16, tag="whhb")
            nc.sync.dma_start(out=bl_sb[:], in_=xpack[:, :])
            nc.sync.dma_start(out=wh_sb[:], in_=whhb[:, :])
            xbig = bl_sb[:, 0:2 * T * BS].rearrange("p (a w) -> p a w", a=2)
            wp_sb = bl_sb[:, 2 * T * BS:]
            wih0 = wp_sb[:, 0:3 * H]
            wih1 = wp_sb[:, 3 * H:6 * H]
            bA_sb = wp_sb[0:4, 6 * H:6 * H + GFD]
            bB_sb = wp_sb[0:4, 6 * H + GFD:6 * H + GFD + HC]
            i4_sb = wp_sb[0:4, 6 * H + GFD + HC:6 * H + GFD + HC + 128]
            whh_rz = [wh_sb[:, (2 * HC) * NG * c:(2 * HC) * NG * (c + 1)]
                      for c in range(4)]
            whh_n = [wh_sb[:, WHH_RZ + HC * NG * c:WHH_RZ + HC * NG * (c + 1)]
                     for c in range(4)]
            id_bf = wh_sb[:, WHH_RZ + WHH_N:WHH_RZ + WHH_N + 128]

            # ---- persistent state (parity-indexed) ----
            hbf = [spool.tile([128, HC], BF16, tag=f"hbf{p}", name=f"hbf{p}")
                   for p in range(2)]
            hT_sb = [spool.tile([128, 128], BF16, tag=f"hT{p}", name=f"hT{p}")
                     for p in range(2)]
            # t=0 reads parity 1 (h(-1) == 0)
            nc.vector.memset(hbf[1][:], 0.0)
            nc.vector.memset(hT_sb[1][:], 0.0)

            # prime the sigmoid/tanh table set before the loop
            prime = wpool.tile([128, 2], F32, tag="prime")
            nc.scalar.activation(prime[:, 0:1], wp_sb[:, 0:1], sig)
            nc.scalar.activation(prime[:, 1:2], prime[:, 0:1], tanh)

            psum_tiles = {}  # step -> (pA, pB), filled LOOKAHEAD early

            def emit_xside(s, xc0, xc1):
                """Bias + x-side matmuls filling psum banks of step s."""
                pA = papool.tile([128, GFD], F32, tag="pA")
                pB = pbpool.tile([128, HC], F32, tag="pB")
                psum_tiles[s] = (pA, pB)
                sl = bass.ts(s, BS)
                nc.tensor.matmul(pA[:, :], i4_sb, bA_sb,
                                 start=True, stop=False, tile_position=(0, 0),
                                 skip_group_check=True)
                nc.tensor.matmul(pB[:, :], i4_sb, bB_sb,
                                 start=True, stop=False, tile_position=(0, 0),
                                 skip_group_check=True)
                for j in range(NG):
                    o = slice(32 * j, 32 * j + BS)
                    g0 = j * GFD
                    nc.tensor.matmul(pA[o, :], xc0[:, sl],
                                     wih0[:, g0:g0 + GFD],
                                     start=False, stop=False,
                                     tile_position=(0, 32 * j),
                                     skip_group_check=True)
                    nc.tensor.matmul(pA[o, :], xc1[:, sl],
                                     wih1[:, g0:g0 + GFD],
                                     start=False, stop=False,
                                     tile_position=(0, 32 * j),
                                     skip_group_check=True)

            def step(s, prefill):
                """Emit one timestep; prefill emits step s+LOOKAHEAD's
                x-side matmuls right after the recurrent waves."""
                p = s % 2
                pA, pB = psum_tiles.pop(s)

                # --- PE prefill first in program order: lands in the
                # previous step's late tail window ---
                if prefill is not None:
                    emit_xside(*prefill)

                # --- transpose h(s-1) -> hT (PE), copy to SBUF ---
                pT = ptpool.tile([128, 128], BF16, tag="pT")
                nc.tensor.transpose(pT[:, :], hbf[1 - p][:, :], id_bf)
                nc.vector.tensor_copy(hT_sb[1 - p][:, :], pT[:, :])

                # --- recurrent waves: r|z first (bank A), then hn (bank B) ---
                for c in range(4):
                    for j in range(NG):
                        oo = slice(32 * j, 32 * (j + 1))
                        nc.tensor.matmul(
                            pA[oo, 0:2 * HC],
                            hT_sb[1 - p][:, 32 * c:32 * (c + 1)],
                            whh_rz[c][:, j * 2 * HC:(j + 1) * 2 * HC],
                            start=False, stop=(c == 3 and j == NG - 1),
                            tile_position=(0, 32 * j),
                            skip_group_check=True)
                for c in range(4):
                    for j in range(NG):
                        oo = slice(32 * j, 32 * (j + 1))
                        nc.tensor.matmul(
                            pB[oo, :],
                            hT_sb[1 - p][:, 32 * c:32 * (c + 1)],
                            whh_n[c][:, j * HC:(j + 1) * HC],
                            start=False, stop=(c == 3 and j == NG - 1),
                            tile_position=(0, 32 * j),
                            skip_group_check=True)

                # --- dummy matmuls: keep the PE busy through the tail so
                # the HAM clock gate stays at 8/8 (transposes don't count
                # as PE-busy; only real matmuls do). Results are junk and
                # never read; they cycle the pT psum slots. Rotate across
                # column strips to avoid piling on strip q0.
                for k in range(NDUM):
                    jd = (s + k) % 4
                    dmy = ptpool.tile([128, 512], F32, tag="pT")
                    nc.tensor.matmul(dmy[32 * jd:32 * (jd + 1), :],
                                     wh_sb[:, 32 * jd:32 * (jd + 1)],
                                     wh_sb[:, 0:512],
                                     start=True, stop=True,
                                     tile_position=(0, 32 * jd),
                                     skip_group_check=True)

                # --- elementwise tail ---
                sr = wpool.tile([128, HC], BF16, tag="sr")
                sz = wpool.tile([128, HC], BF16, tag="sz")
                xn = wpool.tile([128, HC], BF16, tag="xn")
                m = wpool.tile([128, HC], BF16, tag="m")
                a = wpool.tile([128, HC], BF16, tag="a")
                n_t = wpool.tile([128, HC], BF16, tag="n")
                q = wpool.tile([128, HC], BF16, tag="q")
                nwn = wpool.tile([128, HC], BF16, tag="nwn")

                nc.scalar.activation(sr[:], pA[:, 0:HC], sig)
                # evacuate xn to SBUF on the idle ScalarE so `a` runs at
                # DVE bf16 2x rate instead of a 1x psum read
                nc.scalar.copy(xn[:], pA[:, 2 * HC:GFD])
                nc.scalar.activation(sz[:], pA[:, HC:2 * HC], sig)
                mi = nc.vector.tensor_tensor(m[:], sr[:], pB[:, 0:HC],
                                             mybir.AluOpType.mult)
                ai = nc.vector.tensor_tensor(a[:], m[:], xn[:],
                                             mybir.AluOpType.add)
                nc.scalar.activation(n_t[:], a[:], tanh)
                # q = z * h(s-1) runs on DVE during the tanh; the explicit
                # dep keeps the scheduler from hoisting it ahead of m/a
                qi = nc.vector.tensor_tensor(q[:], sz[:], hbf[1 - p][:, :],
                                             mybir.AluOpType.mult)
                add_dep_helper(qi.ins, ai.ins, sync=False,
                               reason="q runs after a")
                # nwn = (z - 1) * n ; h' = q - nwn = z*h + (1-z)*n
                nc.vector.scalar_tensor_tensor(nwn[:], sz[:], 1.0, n_t[:],
                                               mybir.AluOpType.subtract,
                                               mybir.AluOpType.mult)
                nc.vector.tensor_tensor(hbf[p][:, :], q[:], nwn[:],
                                        mybir.AluOpType.subtract)

            if T == CH:
                xc01 = (xbig[:, 0, 0:CH * BS], xbig[:, 1, 0:CH * BS])
                for s in range(LOOKAHEAD):
                    emit_xside(s, *xc01)
                for s in range(CH):
                    pre = ((s + LOOKAHEAD, *xc01)
                           if s + LOOKAHEAD < CH else None)
                    step(s, pre)
            else:
                with tc.For_i(0, T * BS, CH * BS,
                              hint_engines=tuple(mybir.ALL_ENGINES)) as iv:
                    # chunk copy resolves the dynamic offset (ldweights
                    # cannot take register offsets)
                    xc = xpool.tile([128, 2, CH * BS], F16, tag="xc")
                    nc.vector.tensor_copy(
                        xc[:, :, :], xbig[:, :, bass.ds(iv, CH * BS)])
                    xc01 = (xc[:, 0, :], xc[:, 1, :])
                    # steps 0..LOOKAHEAD-1 get their x matmuls up front
                    for s in range(LOOKAHEAD):
                        emit_xside(s, *xc01)
                    for s in range(CH):
                        pre = ((s + LOOKAHEAD, *xc01)
                               if s + LOOKAHEAD < CH else None)
                        step(s, pre)

            # final h lives in hbf[(T-1) % 2]; cast to f32 and store
            hf = wpool.tile([112, HC], F32, tag="hf")
            nc.vector.tensor_copy(hf[:, :], hbf[(T - 1) % 2][0:112, :])
            nc.sync.dma_start(out=hout[:, :], in_=hf[:, :])

    _split_sync_waits(nc)
    return nc


def _split_sync_waits(nc):
    """Walrus codegen allows exactly ONE sync wait per instruction (the TPB
    events struct has a single wait slot). Tile emits multi-wait
    instructions (loop back-edge drains, barrier NoOps, cross-engine RAW
    joins); split the extras onto same-engine NoOps inserted immediately
    before -- the sequencer processes them in order, so semantics are
    identical."""
    for blk in nc.m.functions[0].blocks:
        i = 0
        while i < len(blk.instructions):
            inst = blk.instructions[i]
            si = getattr(inst, "sync_info", None)
            if si and si.on_wait and len(si.on_wait) > 1:
                waits = list(si.on_wait)
                si.on_wait = [waits[-1]]
                for w in waits[:-1]:
                    nop = mybir.InstNoOp(
                        name=nc.get_next_instruction_name(), ins=[], outs=[])
                    nop.engine = inst.engine
                    nop.sync_info = mybir.SyncInfo(on_wait=[w], on_update=[])
                    nc.register_instruction(nop)
                    blk.instructions.insert(i, nop)
                    i += 1
            i += 1


_NC_CACHE = {}


def run(x, W_ih, W_hh, b_ih, b_hh, trace=False):
    from concourse.bass_utils import run_bass_kernel_spmd

    x = np.asarray(x, dtype=np.float32)
    W_ih = np.asarray(W_ih, dtype=np.float32)
    W_hh = np.asarray(W_hh, dtype=np.float32)
    b_ih = np.asarray(b_ih, dtype=np.float32)
    b_hh = np.asarray(b_hh, dtype=np.float32)

    key = (x.shape[1],)
    if key not in _NC_CACHE:
        _NC_CACHE[key] = build_kernel(T=x.shape[1])
    nc = _NC_CACHE[key]

    wts = host_prepare_weights(W_ih, W_hh, b_ih, b_hh)
    in_maps = [{"xpack": host_blob(x, wts["wpack"], c), "whhb": wts["whhb"]}
               for c in range(NCORES)]
    res = run_bass_kernel_spmd(nc, in_maps, list(range(NCORES)), trace=trace)
    h = np.zeros((B, H), np.float32)
    for c in range(NCORES):
        h[c * BS:(c + 1) * BS] = host_post(np.asarray(res.results[c]["hout"]))
    return h, res


def kernel(x, W_ih, W_hh, b_ih, b_hh):
    h, _ = run(x, W_ih, W_hh, b_ih, b_hh)
    return h
